# revision 6
# baseline (speedup 1.0000x reference)
"""Trainium2 Bass kernel for nn_Kmeans (vq_codebook).

Reference semantics (per head h):
    xn    = x / max(||x||_2, 1e-12)            # x: [b, h, l, d]
    dists = einsum('bhld,hcd->bhlc', xn, means)
    buckets = argmax(dists, -1)
    loss  = mean((xn - means[h, buckets])**2) * 1e-4

Sharding: head-parallel over the 8 NeuronCores (one head per core, SPMD —
all ops are independent per head).  Each core computes its full
[b*l, c] = [16384, 512] dists slab plus the per-token top-8 max values and
argmax indices (for the loss).  The host reassembles dists and computes the
scalar loss from (maxval, argmax) using:

    sum_d (xn_d - m_d)^2 = ||xn||^2 - 2*max_c dists + ||m_bucket||^2
                         = 1 - 2*maxval + msq[bucket]

Device data flow per core (tokens tiled 128 at a time, 128 tiles):
    DMA x -> SBUF [128p, tile, d]                     (gpsimd / SWDGE)
    ssq   = reduce_sum(x^2) per token                 (ACT square + DVE reduce)
    norm  = sqrt(ssq) (ACT LUT) + 2 Newton steps      (DVE, full f32 accuracy)
    inv   = 1 / max(norm, eps)                        (DVE reciprocal)
    xT    = PE transpose(x_tile)   [d=64, 128 tok]    (TensorE + ACT copy)
    psum  = xT.T @ meansT          [128 tok, 512 c]   (TensorE, fp32)
    dists = psum * inv[token]      (PSUM->SBUF)       (ACT copy-with-scale)
    maxv8 = top8(dists), idx8 = argmax                (DVE max / max_index)
    DMA dists -> HBM                                  (sync / HWDGE)
"""

import os
import sys

import numpy as np

for _p in ("/root/.axon_site/_ro/trn_rl_repo", "/opt/trn_rl_repo"):
    if os.path.isdir(_p) and _p not in sys.path:
        sys.path.insert(0, _p)

import concourse.bacc as bacc
import concourse.masks as masks
import concourse.tile as tile
from concourse import mybir
from concourse.bass_utils import run_bass_kernel_spmd

B, H, L, D, C = 4, 8, 4096, 64, 512
T = B * L                  # tokens per core (one head per core)
P = 128                    # SBUF partitions / tokens per tile
COMMITMENT = 1e-4
EPS = 1e-12

F32 = mybir.dt.float32
U32 = mybir.dt.uint32


def build_program(n_tiles=T // P, chunk=16, num_devices=8):
    """Build the single-core Bass program (run SPMD on all cores)."""
    t = n_tiles * P  # tokens this program handles
    nc = bacc.Bacc("TRN2", debug=False, enable_asserts=False,
                   num_devices=num_devices)

    x_d = nc.dram_tensor("x", [t, D], F32, kind="ExternalInput").ap()
    m_d = nc.dram_tensor("means", [C, D], F32, kind="ExternalInput").ap()
    dists_d = nc.dram_tensor("dists", [t, C], F32, kind="ExternalOutput").ap()
    maxv_d = nc.dram_tensor("maxv8", [P, n_tiles * 8], F32,
                            kind="ExternalOutput").ap()
    idx_d = nc.dram_tensor("idx8", [P, n_tiles * 8], U32,
                           kind="ExternalOutput").ap()

    with tile.TileContext(nc) as tc:
        _body(tc, x_d, m_d, dists_d, maxv_d, idx_d, n_tiles, chunk)
    nc.compile()
    return nc


def _body(tc, x_d, m_d, dists_d, maxv_d, idx_d, n_tiles, chunk):
    nc = tc.nc
    n_chunks = (n_tiles + chunk - 1) // chunk

    from contextlib import ExitStack
    ctx = ExitStack()
    const = ctx.enter_context(tc.tile_pool(name="const", bufs=1))
    persist = ctx.enter_context(tc.tile_pool(name="persist", bufs=1))
    xsqp = ctx.enter_context(tc.tile_pool(name="xsq", bufs=2))
    xnTp = ctx.enter_context(tc.tile_pool(name="xnT", bufs=4))
    dsbp = ctx.enter_context(tc.tile_pool(name="dsb", bufs=3))
    psumT = ctx.enter_context(tc.tile_pool(name="psumT", bufs=3, space="PSUM"))
    psumD = ctx.enter_context(tc.tile_pool(name="psumD", bufs=5, space="PSUM"))

    ident = const.tile([P, P], F32)
    masks.make_identity(nc, ident[:])

    # ---- means^T [64, 512] ----
    m_sb = const.tile([P, C // P, D], F32)
    nc.gpsimd.dma_start(m_sb[:], m_d.rearrange("(j p) d -> p j d", p=P))
    meansT = const.tile([D, C], F32)
    for j in range(C // P):
        mp = psumT.tile([D, P], F32, tag="pt")
        nc.tensor.transpose(mp[:], m_sb[:, j, :], ident[:])
        nc.scalar.copy(meansT[:, j * P:(j + 1) * P], mp[:])

    # ---- x load + per-token 1/max(||x||, eps) ----
    x_sb = persist.tile([P, n_tiles, D], F32)
    ss = persist.tile([P, n_tiles], F32)     # sum of squares
    nA = persist.tile([P, n_tiles], F32)     # newton scratch
    nB = persist.tile([P, n_tiles], F32)
    rc = persist.tile([P, n_tiles], F32)
    inv = persist.tile([P, n_tiles], F32)
    x_r = x_d.rearrange("(i p) d -> p i d", p=P)

    for j in range(n_chunks):
        lo, hi = j * chunk, min((j + 1) * chunk, n_tiles)
        sl = slice(lo, hi)
        nc.gpsimd.dma_start(x_sb[:, sl, :], x_r[:, sl, :])
        xsq = xsqp.tile([P, chunk, D], F32, tag="xsq")
        w = hi - lo
        nc.scalar.square(xsq[:, :w, :], x_sb[:, sl, :])
        nc.vector.reduce_sum(ss[:, sl], xsq[:, :w, :], axis=mybir.AxisListType.X)
        # norm = sqrt(ss): ACT sqrt seed + 2 Newton iterations (sqrt LUT has a
        # loose precision budget; Newton brings it to ~1 ulp).
        nc.scalar.sqrt(nA[:, sl], ss[:, sl])
        for (src, dst) in ((nA, nB), (nB, nA)):
            nc.vector.reciprocal(rc[:, sl], src[:, sl])
            nc.vector.tensor_mul(rc[:, sl], ss[:, sl], rc[:, sl])
            nc.vector.tensor_add(dst[:, sl], src[:, sl], rc[:, sl])
            nc.vector.tensor_scalar_mul(dst[:, sl], dst[:, sl], 0.5)
        nc.vector.tensor_scalar_max(nA[:, sl], nA[:, sl], EPS)
        nc.vector.reciprocal(inv[:, sl], nA[:, sl])

    # ---- persistent outputs for the loss ----
    maxv = persist.tile([P, n_tiles * 8], F32)
    idx8 = persist.tile([P, n_tiles * 8], U32)

    # ---- main loop over token tiles ----
    for i in range(n_tiles):
        pt = psumT.tile([D, P], F32, tag="pt")
        nc.tensor.transpose(pt[:], x_sb[:, i, :], ident[:])
        xnT = xnTp.tile([D, P], F32, tag="xnT")
        nc.scalar.copy(xnT[:], pt[:])

        pd = psumD.tile([P, C], F32, tag="pd")
        nc.tensor.matmul(pd[:], xnT[:], meansT[:], start=True, stop=True)

        dsb = dsbp.tile([P, C], F32, tag="dsb")
        nc.scalar.mul(dsb[:], pd[:], inv[:, i:i + 1])

        nc.vector.max(maxv[:, 8 * i:8 * i + 8], dsb[:])
        nc.vector.max_index(idx8[:, 8 * i:8 * i + 8],
                            maxv[:, 8 * i:8 * i + 8], dsb[:])

        nc.sync.dma_start(dists_d[P * i:P * (i + 1), :], dsb[:])

    nc.sync.dma_start(maxv_d, maxv[:])
    nc.sync.dma_start(idx_d, idx8[:])
    ctx.close()


_PROGRAM_CACHE = {}


def _get_program():
    key = "full"
    if key not in _PROGRAM_CACHE:
        _PROGRAM_CACHE[key] = build_program()
    return _PROGRAM_CACHE[key]


def kernel(x: np.ndarray, means: np.ndarray):
    """x: [4, 8, 4096, 64] f32, means: [8, 512, 64] f32 ->
    (dists [4, 8, 4096, 512] f32, loss scalar f32)."""
    assert x.shape == (B, H, L, D) and means.shape == (H, C, D)
    nc = _get_program()

    in_maps = []
    for h in range(H):
        x_h = np.ascontiguousarray(
            x[:, h].reshape(T, D).astype(np.float32, copy=False))
        m_h = np.ascontiguousarray(means[h].astype(np.float32, copy=False))
        in_maps.append({"x": x_h, "means": m_h})

    trace = bool(os.environ.get("BASS_TRACE"))
    if trace:
        # The NTFF hook is optional infrastructure; never let profiling
        # break the run.
        try:
            try:
                from antenv import axon_hooks
            except ImportError:
                # antenv in this image lacks the axon_hooks submodule;
                # synthesize it so bass_utils' import finds a registry.
                import types
                import antenv
                axon_hooks = types.ModuleType("antenv.axon_hooks")
                axon_hooks._h = None
                axon_hooks.set_axon_ntff_profile_hook = (
                    lambda h: setattr(axon_hooks, "_h", h))
                axon_hooks.get_axon_ntff_profile_hook = (
                    lambda: axon_hooks._h)
                sys.modules["antenv.axon_hooks"] = axon_hooks
                antenv.axon_hooks = axon_hooks
            if axon_hooks.get_axon_ntff_profile_hook() is None:
                from trn_agent_boot.trn_boot import _ntff_profile_via_ctypes
                axon_hooks.set_axon_ntff_profile_hook(
                    _ntff_profile_via_ctypes("/opt/axon/libaxon_pjrt.so"))
        except Exception as e:  # pragma: no cover
            print(f"NTFF profiling unavailable ({e}); running untraced")
            trace = False

    res = run_bass_kernel_spmd(nc, in_maps, list(range(H)), trace=trace)
    results = res.results

    dists = np.empty((B, H, L, C), dtype=np.float32)
    total = 0.0
    n_tiles = T // P
    for h in range(H):
        out = results[h]
        dists[:, h] = out["dists"].reshape(B, L, C)
        # token t = 128*i + p lives at [p, 8*i] of the top-8 buffers
        top1 = out["maxv8"][:, ::8].T.reshape(-1).astype(np.float64)
        top1_idx = out["idx8"][:, ::8].T.reshape(-1).astype(np.int64)
        msq = (means[h].astype(np.float64) ** 2).sum(-1)
        total += (1.0 - 2.0 * top1 + msq[top1_idx]).sum()

    loss = np.float32(total / (B * H * L * D) * COMMITMENT)
    if hasattr(res, "exec_time_ns") and res.exec_time_ns:
        print(f"HW exec time: {res.exec_time_ns} ns")
    return dists, np.asarray(loss)


# revision 8
# speedup vs baseline: 1.8828x; 1.8828x over previous
"""Trainium2 Bass kernel for nn_Kmeans (vq_codebook).

Reference semantics (per head h):
    xn    = x / max(||x||_2, 1e-12)            # x: [b, h, l, d]
    dists = einsum('bhld,hcd->bhlc', xn, means)
    buckets = argmax(dists, -1)
    loss  = mean((xn - means[h, buckets])**2) * 1e-4

Sharding: head-parallel over the 8 NeuronCores (one head per core, SPMD).

Matmul strategy (pseudo-fp32): split both operands into bf16 hi+lo parts
(x = xh + xl, m = mh + ml) and stack the contraction dim:
    lhsT = [xh^T ; xl^T]  (K=128: rows 0-63 xh, 64-127 xl, bf16)
    MM1: rhs = [mh^T ; mh^T]  -> xh.mh + xl.mh   (start=True)
    MM2: rhs = [ml^T ; 0   ]  -> + xh.ml          (accumulate)
Only the xl.ml term (~2^-18 relative) is dropped; bf16 products are exact in
the fp32 PSUM accumulator.  This runs at bf16 rate (1 cycle/row) instead of
fp32's two half-speed passes — ~4x less TensorE time.

Per-token normalization is folded into the PSUM->SBUF evacuation on ScalarE
(activation Copy with a per-partition scale = 1/max(||x||,eps), where the
norm is computed from a sum-of-squares with an ACT-sqrt seed + one Newton
step + DVE reciprocal for full f32 accuracy).

For the loss, the device emits a grouped max (16 groups of 32 per token,
one DVE tensor_reduce per tile); the host takes the argmax of the 16 group
maxes and finishes the 32-wide argmax on the dists output it already holds,
then computes  loss = mean(1 - 2*max_c dists + ||m_bucket||^2) * 1e-4.
"""

import os
import sys

import numpy as np

for _p in ("/root/.axon_site/_ro/trn_rl_repo", "/opt/trn_rl_repo"):
    if os.path.isdir(_p) and _p not in sys.path:
        sys.path.insert(0, _p)

import concourse.bacc as bacc
import concourse.masks as masks
import concourse.tile as tile
from concourse import mybir
from concourse.bass_utils import run_bass_kernel_spmd

B, H, L, D, C = 4, 8, 4096, 64, 512
T = B * L                  # tokens per core (one head per core)
P = 128                    # SBUF partitions / tokens per tile
G, GW = 16, 32             # loss groups per token: 16 groups of width 32
COMMITMENT = 1e-4
EPS = 1e-12

F32 = mybir.dt.float32
BF16 = mybir.dt.bfloat16


def build_program(n_tiles=T // P, chunk=16, num_devices=8):
    """Build the single-core Bass program (run SPMD on all cores)."""
    t = n_tiles * P
    nc = bacc.Bacc("TRN2", debug=False, enable_asserts=False,
                   num_devices=num_devices)

    x_d = nc.dram_tensor("x", [t, D], F32, kind="ExternalInput").ap()
    m_d = nc.dram_tensor("means", [C, D], F32, kind="ExternalInput").ap()
    dists_d = nc.dram_tensor("dists", [t, C], F32, kind="ExternalOutput").ap()
    gm_d = nc.dram_tensor("gm", [P, n_tiles * G], F32,
                          kind="ExternalOutput").ap()

    with tile.TileContext(nc) as tc:
        _body(tc, x_d, m_d, dists_d, gm_d, n_tiles, chunk)
    nc.compile()
    return nc


def _body(tc, x_d, m_d, dists_d, gm_d, n_tiles, chunk):
    nc = tc.nc
    n_chunks = (n_tiles + chunk - 1) // chunk

    from contextlib import ExitStack
    ctx = ExitStack()
    const = ctx.enter_context(tc.tile_pool(name="const", bufs=1))
    persist = ctx.enter_context(tc.tile_pool(name="persist", bufs=1))
    xsqp = ctx.enter_context(tc.tile_pool(name="xsq", bufs=2))
    xhlp = ctx.enter_context(tc.tile_pool(name="xhl", bufs=4))
    lhsp = ctx.enter_context(tc.tile_pool(name="lhs", bufs=4))
    dsbp = ctx.enter_context(tc.tile_pool(name="dsb", bufs=4))
    psumT = ctx.enter_context(tc.tile_pool(name="psumT", bufs=3, space="PSUM"))
    psumD = ctx.enter_context(tc.tile_pool(name="psumD", bufs=5, space="PSUM"))

    ident = const.tile([P, P], BF16)
    masks.make_identity(nc, ident[:])

    # ---- means: build stacked bf16 hi/lo rhs operands ----
    # mm1_rhs [128, 512] = [mh^T ; mh^T], mm2_rhs = [ml^T ; 0]
    m_sb = const.tile([P, C // P, D], F32)
    nc.gpsimd.dma_start(m_sb[:], m_d.rearrange("(j p) d -> p j d", p=P))
    mm1_rhs = const.tile([P, C], BF16)
    mm2_rhs = const.tile([P, C], BF16)
    for j in range(C // P):
        # stacked-source tiles: [mh | mh] and [ml | 0] (all free-dim writes)
        mh = const.tile([P, 2, D], BF16, tag="mstage")
        nc.scalar.copy(mh[:, 0, :], m_sb[:, j, :])
        nc.vector.tensor_sub(mh[:, 1, :], m_sb[:, j, :], mh[:, 0, :])
        mz = const.tile([P, 2, D], BF16, tag="mstage2")
        nc.vector.tensor_copy(mz[:, 0, :], mh[:, 1, :])   # ml
        nc.vector.memset(mz[:, 1, :], 0.0)
        nc.vector.tensor_copy(mh[:, 1, :], mh[:, 0, :])   # duplicate mh
        pt1 = psumT.tile([P, P], BF16, tag="pt")
        nc.tensor.transpose(pt1[:], mh[:], ident[:])
        nc.scalar.copy(mm1_rhs[:, j * P:(j + 1) * P], pt1[:])
        pt2 = psumT.tile([P, P], BF16, tag="pt")
        nc.tensor.transpose(pt2[:], mz[:], ident[:])
        nc.scalar.copy(mm2_rhs[:, j * P:(j + 1) * P], pt2[:])

    # ---- x load + per-token 1/max(||x||, eps) ----
    x_sb = persist.tile([P, n_tiles, D], F32)
    ss = persist.tile([P, n_tiles], F32)
    nA = persist.tile([P, n_tiles], F32)
    nB = persist.tile([P, n_tiles], F32)
    rc = persist.tile([P, n_tiles], F32)
    inv = persist.tile([P, n_tiles], F32)
    x_r = x_d.rearrange("(i p) d -> p i d", p=P)

    for j in range(n_chunks):
        lo, hi = j * chunk, min((j + 1) * chunk, n_tiles)
        sl = slice(lo, hi)
        nc.gpsimd.dma_start(x_sb[:, sl, :], x_r[:, sl, :])
        xsq = xsqp.tile([P, chunk, D], F32, tag="xsq")
        w = hi - lo
        nc.scalar.square(xsq[:, :w, :], x_sb[:, sl, :])
        nc.vector.reduce_sum(ss[:, sl], xsq[:, :w, :], axis=mybir.AxisListType.X)
        # norm = sqrt(ss): ACT sqrt seed + one Newton step (-> ~1e-6 rel)
        nc.scalar.sqrt(nA[:, sl], ss[:, sl])
        nc.vector.reciprocal(rc[:, sl], nA[:, sl])
        nc.vector.tensor_mul(rc[:, sl], ss[:, sl], rc[:, sl])
        nc.vector.tensor_add(nB[:, sl], nA[:, sl], rc[:, sl])
        nc.vector.tensor_scalar_mul(nB[:, sl], nB[:, sl], 0.5)
        nc.vector.tensor_scalar_max(nB[:, sl], nB[:, sl], EPS)
        nc.vector.reciprocal(inv[:, sl], nB[:, sl])

    gmb = persist.tile([P, n_tiles * G], F32)

    # ---- main loop over token tiles ----
    for i in range(n_tiles):
        # bf16 hi/lo split, stacked [xh | xl] then transposed -> [xh^T; xl^T]
        xhl = xhlp.tile([P, 2, D], BF16, tag="xhl")
        nc.vector.tensor_copy(xhl[:, 0, :], x_sb[:, i, :])
        nc.vector.tensor_sub(xhl[:, 1, :], x_sb[:, i, :], xhl[:, 0, :])

        pt = psumT.tile([P, P], BF16, tag="pt")
        nc.tensor.transpose(pt[:], xhl[:], ident[:])
        lhsT = lhsp.tile([P, P], BF16, tag="lhsT")
        nc.scalar.copy(lhsT[:], pt[:])

        pd = psumD.tile([P, C], F32, tag="pd")
        nc.tensor.matmul(pd[:], lhsT[:], mm1_rhs[:], start=True, stop=False)
        nc.tensor.matmul(pd[:], lhsT[:], mm2_rhs[:], start=False, stop=True)

        dsb = dsbp.tile([P, C], F32, tag="dsb")
        nc.scalar.mul(dsb[:], pd[:], inv[:, i:i + 1])

        nc.vector.reduce_max(gmb[:, G * i:G * (i + 1)],
                             dsb[:].rearrange("p (g w) -> p g w", g=G),
                             axis=mybir.AxisListType.X)

        nc.sync.dma_start(dists_d[P * i:P * (i + 1), :], dsb[:])

    nc.sync.dma_start(gm_d, gmb[:])
    ctx.close()


_PROGRAM_CACHE = {}


def _get_program():
    key = "full"
    if key not in _PROGRAM_CACHE:
        _PROGRAM_CACHE[key] = build_program()
    return _PROGRAM_CACHE[key]


def kernel(x: np.ndarray, means: np.ndarray):
    """x: [4, 8, 4096, 64] f32, means: [8, 512, 64] f32 ->
    (dists [4, 8, 4096, 512] f32, loss scalar f32)."""
    assert x.shape == (B, H, L, D) and means.shape == (H, C, D)
    nc = _get_program()

    in_maps = []
    for h in range(H):
        x_h = np.ascontiguousarray(
            x[:, h].reshape(T, D).astype(np.float32, copy=False))
        m_h = np.ascontiguousarray(means[h].astype(np.float32, copy=False))
        in_maps.append({"x": x_h, "means": m_h})

    trace = bool(os.environ.get("BASS_TRACE"))
    if trace:
        # The NTFF hook is optional infrastructure; never let profiling
        # break the run.
        try:
            try:
                from antenv import axon_hooks
            except ImportError:
                import types
                import antenv
                axon_hooks = types.ModuleType("antenv.axon_hooks")
                axon_hooks._h = None
                axon_hooks.set_axon_ntff_profile_hook = (
                    lambda h: setattr(axon_hooks, "_h", h))
                axon_hooks.get_axon_ntff_profile_hook = (
                    lambda: axon_hooks._h)
                sys.modules["antenv.axon_hooks"] = axon_hooks
                antenv.axon_hooks = axon_hooks
            if axon_hooks.get_axon_ntff_profile_hook() is None:
                from trn_agent_boot.trn_boot import _ntff_profile_via_ctypes
                axon_hooks.set_axon_ntff_profile_hook(
                    _ntff_profile_via_ctypes("/opt/axon/libaxon_pjrt.so"))
        except Exception as e:  # pragma: no cover
            print(f"NTFF profiling unavailable ({e}); running untraced")
            trace = False

    res = run_bass_kernel_spmd(nc, in_maps, list(range(H)), trace=trace)
    results = res.results

    dists = np.empty((B, H, L, C), dtype=np.float32)
    total = 0.0
    n_tiles = T // P
    for h in range(H):
        out = results[h]
        d_h = out["dists"]                      # [T, C]
        dists[:, h] = d_h.reshape(B, L, C)
        # token t = 128*i + p lives at gm[p, 16*i : 16*i+16]
        gm = out["gm"].reshape(P, n_tiles, G).transpose(1, 0, 2).reshape(T, G)
        g = gm.argmax(-1)                       # winning group per token
        grp = np.take_along_axis(d_h.reshape(T, G, GW), g[:, None, None],
                                 axis=1)[:, 0, :]
        widx = grp.argmax(-1)
        idx = g * GW + widx                     # argmax_c dists
        maxv = np.take_along_axis(grp, widx[:, None], axis=1)[:, 0]
        msq = (means[h].astype(np.float64) ** 2).sum(-1)
        total += (1.0 - 2.0 * maxv.astype(np.float64) + msq[idx]).sum()

    loss = np.float32(total / (B * H * L * D) * COMMITMENT)
    if getattr(res, "exec_time_ns", None):
        print(f"HW exec time: {res.exec_time_ns} ns")
    return dists, np.asarray(loss)
